# revision 15
# baseline (speedup 1.0000x reference)
"""KAN layer kernel for Trainium2, 8 NeuronCores.

out[b,o] = sum_i silu(x)[b,i]*mask[i,o]
         + sum_{i,g} B3[b,i,g] * scale_sp[i,o]*mask[i,o] * coef[i,o,g]

The grid is uniform, so every cubic B-spline basis function is a shifted
copy of one bump: B3[b,i,g] = b3(u - g), u = (x+1.375)/h, h = 0.125.
b3 is evaluated exactly (piecewise cubic) branch-free via
    w  = relu(2 - |u - g - 2|)          (fold + clamp)
    6*b3 = min(w^3, 4 - 3*w*(w-2)^2)
Sharding: contraction dim I=512 split 8 ways (64 rows/core); each core
computes a full (256,512) partial which the host sums.
"""

import numpy as np

B, I, O, G = 256, 512, 512, 19
IS = I // 8          # 64 input dims per core
SLOTS = IS * G       # 1216 spline contraction slots per core
NT = 10              # K tiles of 128 (1280 = SLOTS padded + unused)
PAD_BIAS = 50000.0   # forces w=0 on pad slots
H = 0.125
U_SCALE = 1.0 / H    # 8
# a = |8*x + (9 - g)|  since u-2-g = 8x + 11 - 2 - g

_compiled = {}


def _build_program():
    import concourse.mybir as mybir
    import concourse.tile as tile
    from concourse import bacc

    f32 = mybir.dt.float32
    f16 = mybir.dt.float16
    AF = mybir.ActivationFunctionType
    OP = mybir.AluOpType

    nc = bacc.Bacc(trn_type="TRN2")

    xrep_d = nc.dram_tensor("xrep", [128, NT * B], f32, kind="ExternalInput")
    biasv_d = nc.dram_tensor("biasv", [128, NT], f32, kind="ExternalInput")
    coef_d = nc.dram_tensor("coef_t", [NT * 128, O], f16, kind="ExternalInput")
    scale_d = nc.dram_tensor("scale_rep", [NT * 128, O], f16, kind="ExternalInput")
    maskr_d = nc.dram_tensor("mask_rep", [NT * 128, O], f16, kind="ExternalInput")
    maskh_d = nc.dram_tensor("maskh", [IS, O], f16, kind="ExternalInput")
    xt_d = nc.dram_tensor("xt", [IS, B], f32, kind="ExternalInput")
    out_d = nc.dram_tensor("out_part", [B, O], f32, kind="ExternalOutput")

    FD = NT * B  # 2560

    with tile.TileContext(nc) as tc:
        with (
            tc.tile_pool(name="io", bufs=1) as io,
            tc.tile_pool(name="mid", bufs=1) as mid,
            tc.tile_pool(name="ps", bufs=1, space="PSUM") as psp,
        ):
            xrep = io.tile([128, FD], f32, tag="xrep")
            for t in range(NT):
                nc.sync.dma_start(
                    out=xrep[:, t * B:(t + 1) * B],
                    in_=xrep_d[:, t * B:(t + 1) * B],
                )
            biasv = io.tile([128, NT], f32, tag="biasv")
            nc.sync.dma_start(out=biasv[:], in_=biasv_d[:])
            coef = io.tile([128, NT * O], f16, tag="coef")
            scaler = io.tile([128, NT * O], f16, tag="scaler")
            maskr = io.tile([128, NT * O], f16, tag="maskr")
            for t in range(NT):
                nc.sync.dma_start(
                    out=coef[:, t * O:(t + 1) * O],
                    in_=coef_d[t * 128:(t + 1) * 128, :],
                )
                nc.sync.dma_start(
                    out=scaler[:, t * O:(t + 1) * O],
                    in_=scale_d[t * 128:(t + 1) * 128, :],
                )
                nc.sync.dma_start(
                    out=maskr[:, t * O:(t + 1) * O],
                    in_=maskr_d[t * 128:(t + 1) * 128, :],
                )
            maskh = io.tile([IS, O], f16, tag="maskh")
            nc.sync.dma_start(out=maskh[:], in_=maskh_d[:])
            xt = io.tile([IS, B], f32, tag="xt")
            nc.sync.dma_start(out=xt[:], in_=xt_d[:])

            # ---- s_rep = scale_rep * mask_rep  (fp16, per K tile) ----
            s_rep = mid.tile([128, NT * O], f16, tag="s_rep")
            for t in range(NT):
                nc.vector.tensor_tensor(
                    s_rep[:, t * O:(t + 1) * O],
                    scaler[:, t * O:(t + 1) * O],
                    maskr[:, t * O:(t + 1) * O],
                    op=OP.mult,
                )

            # ---- C = coef * s_rep  (fp16, per K tile) ----
            cmat = mid.tile([128, NT * O], f16, tag="cmat")
            for t in range(NT):
                nc.vector.tensor_tensor(
                    cmat[:, t * O:(t + 1) * O],
                    coef[:, t * O:(t + 1) * O],
                    s_rep[:, t * O:(t + 1) * O],
                    op=OP.mult,
                )

            # ---- basis chain ----
            bias_p2 = mid.tile([128, 1], f32, tag="bias_p2")
            nc.vector.memset(bias_p2[:], 2.0)
            bias_m2 = mid.tile([128, 1], f32, tag="bias_m2")
            nc.vector.memset(bias_m2[:], -2.0)
            # d2 = 8*x + (9 - g), computed on DVE (per-partition bias AP);
            # keeps every ACT instruction at <=1 semaphore wait (AC struct
            # only encodes one sync-wait).
            d2_t = mid.tile([128, FD], f32, tag="d2_t")
            for t in range(NT):
                nc.vector.tensor_scalar(
                    d2_t[:, t * B:(t + 1) * B],
                    xrep[:, t * B:(t + 1) * B],
                    U_SCALE,
                    biasv[:, t:t + 1],
                    op0=OP.mult,
                    op1=OP.add,
                )
            a_t = mid.tile([128, FD], f16, tag="a_t")
            nc.scalar.activation(a_t[:], d2_t[:], AF.Abs)
            w_t = mid.tile([128, FD], f16, tag="w_t")
            nc.scalar.activation(w_t[:], a_t[:], AF.Relu, bias=bias_p2[:], scale=-1.0)
            v_t = mid.tile([128, FD], f16, tag="v_t")
            nc.scalar.activation(v_t[:], w_t[:], AF.Square, bias=bias_m2[:], scale=1.0)
            # basis scaled by 1/6 here (min(w^3, 4-3w(w-2)^2)/6):
            # c = (w*w)*(w/6),  q = -0.5*p2 + 2/3
            z_t = mid.tile([128, FD], f16, tag="z_t")
            nc.vector.tensor_tensor(z_t[:], w_t[:], w_t[:], op=OP.mult)
            w6_t = mid.tile([128, FD], f16, tag="w6_t")
            nc.vector.tensor_scalar(w6_t[:], w_t[:], 1.0 / 6.0, None, op0=OP.mult)
            c_t = mid.tile([128, FD], f16, tag="c_t")
            nc.vector.tensor_tensor(c_t[:], z_t[:], w6_t[:], op=OP.mult)
            p_t = mid.tile([128, FD], f16, tag="p_t")
            nc.vector.tensor_tensor(p_t[:], w_t[:], v_t[:], op=OP.mult)
            q_t = mid.tile([128, FD], f16, tag="q_t")
            nc.vector.tensor_scalar(
                q_t[:], p_t[:], -0.5, 2.0 / 3.0, op0=OP.mult, op1=OP.add
            )
            bas = mid.tile([128, FD], f16, tag="bas")
            nc.vector.tensor_tensor(bas[:], c_t[:], q_t[:], op=OP.min)

            # ---- base branch: silu(x)^T in fp16 ----
            sg = mid.tile([IS, B], f32, tag="sg")
            nc.scalar.activation(sg[:], xt[:], AF.Sigmoid)
            silu = mid.tile([IS, B], f16, tag="silu")
            nc.vector.tensor_tensor(silu[:], xt[:], sg[:], op=OP.mult)

            # ---- matmuls: accumulate K tiles + base into PSUM ----
            out_sb = mid.tile([128, 2 * O], f32, tag="out_sb")
            for m in range(2):
                ps = psp.tile([128, O], f32, tag=f"ps{m}")
                for t in range(NT):
                    nc.tensor.matmul(
                        ps[:],
                        bas[:, t * B + m * 128: t * B + (m + 1) * 128],
                        cmat[:, t * O:(t + 1) * O],
                        start=(t == 0),
                        stop=False,
                    )
                nc.tensor.matmul(
                    ps[:],
                    silu[:, m * 128:(m + 1) * 128],
                    maskh[:],
                    start=False,
                    stop=True,
                )
                nc.scalar.copy(out_sb[:, m * O:(m + 1) * O], ps[:])
                nc.sync.dma_start(
                    out=out_d[m * 128:(m + 1) * 128, :],
                    in_=out_sb[:, m * O:(m + 1) * O],
                )
    nc.compile()
    return nc


def _prep_core(c, x, coef, scale_sp, mask):
    i0 = c * IS
    xs = x[:, i0:i0 + IS]                      # (B, IS)
    slots = np.arange(NT * 128)
    ii = np.minimum(slots // G, IS - 1)
    gg = slots % G
    valid = slots < SLOTS

    xr = xs.T[ii].astype(np.float32)           # (1280, B)
    xr[~valid] = 0.0
    xrep = np.ascontiguousarray(
        xr.reshape(NT, 128, B).transpose(1, 0, 2).reshape(128, NT * B)
    )

    bias = np.where(valid, 9.0 - gg, PAD_BIAS).astype(np.float32)
    biasv = np.ascontiguousarray(bias.reshape(NT, 128).T)

    ct = coef[i0:i0 + IS].transpose(0, 2, 1).reshape(SLOTS, O)
    coef_t = np.zeros((NT * 128, O), np.float16)
    coef_t[:SLOTS] = ct.astype(np.float16)

    # replicate scale/mask rows x19 to match the (i,g) slot layout
    scale_rep = np.ascontiguousarray(scale_sp[i0 + ii]).astype(np.float16)
    mask_rep = np.ascontiguousarray(mask[i0 + ii]).astype(np.float16)

    return {
        "xrep": xrep,
        "biasv": biasv,
        "coef_t": coef_t,
        "scale_rep": scale_rep,
        "mask_rep": mask_rep,
        "maskh": np.ascontiguousarray(mask[i0:i0 + IS]).astype(np.float16),
        "xt": np.ascontiguousarray(xs.T).astype(np.float32),
    }


def kernel(x, grid, coef, scale_sp, mask):
    from concourse.bass_utils import run_bass_kernel_spmd

    x = np.asarray(x, np.float32)
    coef = np.asarray(coef, np.float32)
    scale_sp = np.asarray(scale_sp, np.float32)
    mask = np.asarray(mask, np.float32)

    if "nc" not in _compiled:
        _compiled["nc"] = _build_program()
    nc = _compiled["nc"]

    in_maps = [_prep_core(c, x, coef, scale_sp, mask) for c in range(8)]
    res = run_bass_kernel_spmd(nc, in_maps, list(range(8)))
    parts = [r["out_part"] for r in res.results]
    return np.sum(np.stack(parts), axis=0, dtype=np.float64).astype(np.float32)
